# revision 11
# baseline (speedup 1.0000x reference)
"""Trainium2 Bass kernel for nn_Prompt (retrieval_knn, top_k=1).

Computes, for x_embed [32,4096,768] and wte [500,768]:
  prompt_norm   = l2norm(wte, axis=1)
  x_norm        = l2norm(x_embed, axis=2)
  similarity    = x_norm @ prompt_norm.T          [32,4096,500]
  idx           = argmax(similarity)              [32,4096,1]
  selected      = wte[idx]
  prompted      = selected + x_embed              [32,4096,768]
  bkn           = prompt_norm[idx]                [32,4096,1,768]
  reduce_sim    = sum(similarity at idx) / 32

Sharding: data-parallel over batch, 4 batches (16384 tokens) per core on
8 NeuronCores.  wte-derived tables are replicated.

Device algorithm per 128-token tile:
  - DMA x tile [128,768] (natural layout)
  - PE-transpose to xT (6x 128x128 blocks, fp32) into PSUM
  - DVE: split xT into bf16 hi/lo (xh = bf16(xT), xl = bf16(xT-xh))
  - PE: sim_psum [128,500] = xh@ph + xh@pl + xl@ph (18 bf16 matmuls,
    fp32 PSUM accumulate) -- error ~1e-6, measured 0 argmax flips vs fp32
  - ACT: row sumsq of x (Square+accum), inv_norm = Sqrt(1/ss) via DVE recip
  - ACT: sim = sim_psum * inv_norm (Copy w/ per-partition scale) -> DMA out
  - DVE: max8 + max_index -> idx (uint32) -> int32 -> DMA out
  - SWDGE indirect gather: selected = wte[idx] rows, invw_t = invw[idx]
  - ACT: bkn = selected * invw_t -> DMA out
  - DVE: prompted = selected + x -> DMA out

reduce_sim is assembled on host from similarity+idx (exact same values).
"""

import numpy as np
import ml_dtypes

import concourse.bass as bass
import concourse.mybir as mybir
from concourse import bacc
from concourse.tile import TileContext
from concourse.bass_utils import run_bass_kernel_spmd

B, S, D, V = 32, 4096, 768, 500
N_CORES = 8
TILE = 128
KC = D // 128  # 6 contraction chunks
TOK_PER_CORE = (B // N_CORES) * S  # 16384
FULL_TILES = TOK_PER_CORE // TILE  # 128

F32 = mybir.dt.float32
BF16 = mybir.dt.bfloat16
I32 = mybir.dt.int32
U32 = mybir.dt.uint32


def build_program(n_tiles, work_bufs=6, sim_bufs=4, xt_bufs=2, prom_engine="vector",
                  xh_engine="scalar", ablate=frozenset()):
    ntok = n_tiles * TILE
    nc = bacc.Bacc(
        "TRN2", target_bir_lowering=False, debug=False, num_devices=N_CORES
    )
    x_t = nc.dram_tensor("x", [ntok, D], F32, kind="ExternalInput")
    wte_t = nc.dram_tensor("wte", [V, D], F32, kind="ExternalInput")
    ph_t = nc.dram_tensor("ph", [128, KC * V], BF16, kind="ExternalInput")
    pl_t = nc.dram_tensor("pl", [128, KC * V], BF16, kind="ExternalInput")
    invw_t = nc.dram_tensor("invw", [V, 1], F32, kind="ExternalInput")
    ident_t = nc.dram_tensor("ident", [128, 128], F32, kind="ExternalInput")

    sim_o = nc.dram_tensor("sim", [ntok, V], F32, kind="ExternalOutput")
    idx_o = nc.dram_tensor("idx", [ntok, 1], I32, kind="ExternalOutput")
    prom_o = nc.dram_tensor("prom", [ntok, D], F32, kind="ExternalOutput")
    bkn_o = nc.dram_tensor("bkn", [ntok, D], F32, kind="ExternalOutput")

    x_ap = x_t.ap()
    sim_ap = sim_o.ap()
    idx_ap = idx_o.ap()
    prom_ap = prom_o.ap()
    bkn_ap = bkn_o.ap()

    with TileContext(nc) as tc:
        with (
            tc.tile_pool(name="const", bufs=1) as cpool,
            tc.tile_pool(name="work", bufs=work_bufs) as pool,
            tc.tile_pool(name="psum_sim", bufs=sim_bufs, space="PSUM") as psum_sim,
            tc.tile_pool(name="psum_xt", bufs=xt_bufs, space="PSUM") as psum_xt,
        ):
            ph_sb = cpool.tile([128, KC * V], BF16)
            pl_sb = cpool.tile([128, KC * V], BF16)
            id_sb = cpool.tile([128, 128], F32)
            nc.sync.dma_start(out=ph_sb[:], in_=ph_t.ap())
            nc.sync.dma_start(out=pl_sb[:], in_=pl_t.ap())
            nc.sync.dma_start(out=id_sb[:], in_=ident_t.ap())

            for t in range(n_tiles):
                rows = slice(t * TILE, (t + 1) * TILE)
                xt = pool.tile([128, D], F32, tag="xt")
                nc.sync.dma_start(out=xt[:], in_=x_ap[rows, :])

                # transpose x tile into PSUM (6 fp32 128x128 PE transposes)
                xT_ps = psum_xt.tile([128, D], F32, tag="xT")
                for k in range(KC) if "transpose" not in ablate else []:
                    ks = slice(k * 128, (k + 1) * 128)
                    nc.tensor.transpose(
                        out=xT_ps[:, ks], in_=xt[:, ks], identity=id_sb[:]
                    )

                # bf16 hi/lo split of xT
                xh = pool.tile([128, D], BF16, tag="xh")
                if xh_engine == "scalar":
                    nc.scalar.copy(out=xh[:], in_=xT_ps[:])
                else:
                    nc.vector.tensor_copy(out=xh[:], in_=xT_ps[:])
                xl = pool.tile([128, D], BF16, tag="xl")
                nc.vector.tensor_sub(out=xl[:], in0=xT_ps[:], in1=xh[:])

                # similarity matmuls: sim = xh@ph + xh@pl + xl@ph
                sim_ps = psum_sim.tile([128, V], F32, tag="sim")
                for k in range(KC) if "matmul" not in ablate else [0]:
                    ks = slice(k * 128, (k + 1) * 128)
                    vs = slice(k * V, (k + 1) * V)
                    nc.tensor.matmul(
                        out=sim_ps[:], lhsT=xh[:, ks], rhs=ph_sb[:, vs],
                        start=(k == 0), stop=False,
                    )
                    nc.tensor.matmul(
                        out=sim_ps[:], lhsT=xh[:, ks], rhs=pl_sb[:, vs],
                        start=False, stop=False,
                    )
                    nc.tensor.matmul(
                        out=sim_ps[:], lhsT=xl[:, ks], rhs=ph_sb[:, vs],
                        start=False, stop=(k == KC - 1),
                    )

                # 1/||x|| per row: Square+accum on ACT, recip on DVE, Sqrt on ACT
                sq = pool.tile([128, D], F32, tag="sq")
                ssx = pool.tile([128, 1], F32, tag="ssx")
                nc.scalar.activation(
                    out=sq[:], in_=xt[:],
                    func=mybir.ActivationFunctionType.Square, accum_out=ssx[:],
                )
                rx = pool.tile([128, 1], F32, tag="rx")
                nc.vector.reciprocal(out=rx[:], in_=ssx[:])
                ix = pool.tile([128, 1], F32, tag="ix")
                nc.scalar.activation(
                    out=ix[:], in_=rx[:], func=mybir.ActivationFunctionType.Sqrt
                )

                # scaled similarity -> SBUF -> DRAM
                sim_sb = pool.tile([128, V], F32, tag="simsb")
                nc.scalar.activation(
                    out=sim_sb[:], in_=sim_ps[:],
                    func=mybir.ActivationFunctionType.Copy, scale=ix[:],
                )
                if "simout" not in ablate:
                    nc.sync.dma_start(out=sim_ap[rows, :], in_=sim_sb[:])

                # argmax
                mx8 = pool.tile([128, 8], F32, tag="mx8")
                nc.vector.max(out=mx8[:], in_=sim_sb[:])
                ix8 = pool.tile([128, 8], U32, tag="ix8")
                nc.vector.max_index(out=ix8[:], in_max=mx8[:], in_values=sim_sb[:])
                idxt = pool.tile([128, 1], I32, tag="idxt")
                nc.vector.tensor_copy(out=idxt[:], in_=ix8[:, :1])
                if "idxout" not in ablate:
                    nc.sync.dma_start(out=idx_ap[rows, :], in_=idxt[:])

                # gather wte rows + inv wte norms
                sel = pool.tile([128, D], F32, tag="sel")
                if "selgather" not in ablate:
                    nc.gpsimd.indirect_dma_start(
                        out=sel[:], out_offset=None, in_=wte_t.ap(),
                        in_offset=bass.IndirectOffsetOnAxis(ap=idxt[:, :1], axis=0),
                    )
                iwt = pool.tile([128, 1], F32, tag="iwt")
                if "iwtgather" not in ablate:
                    nc.gpsimd.indirect_dma_start(
                        out=iwt[:], out_offset=None, in_=invw_t.ap(),
                        in_offset=bass.IndirectOffsetOnAxis(ap=idxt[:, :1], axis=0),
                    )
                else:
                    nc.vector.memset(iwt[:], 1.0)

                # bkn = selected * invw[idx]
                bkn = pool.tile([128, D], F32, tag="bkn")
                nc.scalar.activation(
                    out=bkn[:], in_=sel[:],
                    func=mybir.ActivationFunctionType.Copy, scale=iwt[:],
                )
                nc.sync.dma_start(out=bkn_ap[rows, :], in_=bkn[:])

                # prompted = selected + x
                prom = pool.tile([128, D], F32, tag="prom")
                if prom_engine == "gpsimd":
                    nc.gpsimd.scalar_tensor_tensor(
                        out=prom[:], in0=sel[:], scalar=1.0, in1=xt[:],
                        op0=mybir.AluOpType.mult, op1=mybir.AluOpType.add,
                    )
                else:
                    nc.vector.tensor_add(out=prom[:], in0=sel[:], in1=xt[:])
                nc.sync.dma_start(out=prom_ap[rows, :], in_=prom[:])

    nc.compile()
    return nc


_CACHED = {}


def _get_program(n_tiles):
    if n_tiles not in _CACHED:
        _CACHED[n_tiles] = build_program(n_tiles)
    return _CACHED[n_tiles]


def _host_tables(wte):
    wte = np.asarray(wte, dtype=np.float32)
    ss = np.sum(wte * wte, axis=1)
    invw = (1.0 / np.sqrt(ss)).astype(np.float32)
    p = wte * invw[:, None]  # prompt_norm, fp32
    ph32 = p.astype(ml_dtypes.bfloat16).astype(np.float32)
    ph = ph32.astype(ml_dtypes.bfloat16)
    pl = (p - ph32).astype(ml_dtypes.bfloat16)

    def to_chunks(a):  # [V,D] -> [128, KC*V] with chunk k at cols [k*V,(k+1)*V)
        return (
            np.ascontiguousarray(
                a.T.reshape(KC, 128, V).transpose(1, 0, 2).reshape(128, KC * V)
            )
        )

    return to_chunks(ph), to_chunks(pl), invw.reshape(V, 1)


def run_on_device(x_flat, wte, n_tiles, trace=False):
    """x_flat: [N_CORES * n_tiles*128, D] fp32. Returns per-core result dicts."""
    nc = _get_program(n_tiles)
    ph, pl, invw = _host_tables(wte)
    ident = np.eye(128, dtype=np.float32)
    ntok = n_tiles * TILE
    in_maps = []
    for c in range(N_CORES):
        in_maps.append(
            {
                "x": np.ascontiguousarray(x_flat[c * ntok : (c + 1) * ntok]),
                "wte": np.asarray(wte, dtype=np.float32),
                "ph": ph,
                "pl": pl,
                "invw": invw,
                "ident": ident,
            }
        )
    res = run_bass_kernel_spmd(
        nc, in_maps, core_ids=list(range(N_CORES)), trace=trace
    )
    return res


def kernel(x_embed, wte):
    x_embed = np.asarray(x_embed, dtype=np.float32)
    wte = np.asarray(wte, dtype=np.float32)
    x_flat = x_embed.reshape(B * S, D)

    res = run_on_device(x_flat, wte, FULL_TILES)

    sim = np.concatenate([res.results[c]["sim"] for c in range(N_CORES)], 0)
    idx = np.concatenate([res.results[c]["idx"] for c in range(N_CORES)], 0)
    prom = np.concatenate([res.results[c]["prom"] for c in range(N_CORES)], 0)
    bkn = np.concatenate([res.results[c]["bkn"] for c in range(N_CORES)], 0)

    similarity = sim.reshape(B, S, V)
    idx_out = idx.reshape(B, S, 1).astype(np.int32)
    prompted = prom.reshape(B, S, D)
    bkn_out = bkn.reshape(B, S, 1, D)

    maxes = np.take_along_axis(sim, idx.astype(np.int64), axis=1)
    reduce_sim = np.float32(np.sum(maxes.astype(np.float64)) / B)

    return (prompted, similarity, reduce_sim, idx_out, bkn_out)
